# revision 2
# baseline (speedup 1.0000x reference)
"""BinaryConv2D Trainium2 kernel — fp8 DoubleRow edition.

Reference computation:
    out = conv2d(sign(x), sign(w), SAME, stride 1)   # sign(v) = +1 if v>=0 else -1
    x: (64, 56, 56, 128) f32, w: (3, 3, 128, 256) f32 -> out (64, 56, 56, 256) f32

Strategy (data-parallel over batch, 8 images per NeuronCore):
  * Host binarizes x to fp8e4 +-1 bytes and lays it out channel-major into
    zero-padded 58x58 planes (plus 1B/3B guards), so the device does no
    input preprocessing at all: one straight DMA per image into SBUF.
  * Weights are host-binarized to fp8 and packed as 4 DoubleRow tap-pairs
    [ci, pair, k, co] plus one leftover tap. A DoubleRow matmul contracts
    128 channels x 2 taps per pass: lhsT = w-pair [128, 2, 128co], rhs =
    two overlapping shifted windows of the padded plane [128, 2, Npx]
    (overlapping strided APs are legal on the moving operand), out PSUM
    [128co, Npx] f32. 9 taps = 4 DR + 1 normal matmul per 512-px block.
  * Output stays co-major: PSUM -> SBUF f16 (exact: integer sums <= 1152)
    -> HBM scratch [img, half, co128, px]. The host strips row padding,
    transposes to NHWC and upcasts to f32 during unshard.
"""

import sys

if "/opt/trn_rl_repo" not in sys.path:
    sys.path.insert(0, "/opt/trn_rl_repo")

import numpy as np

import concourse.bacc as bacc
import concourse.bass as bass
import concourse.mybir as mybir
from concourse.ap import AP
from concourse.tile import TileContext
from concourse.bass_utils import run_bass_kernel_spmd

N_CORES = 8
IMGS = 8  # images per core
H = W = 56
C = 128  # input channels (contraction partitions)
O = 256  # output channels
PW = 57  # padded row width: [pad, 56 data] — the right pad of row r IS the
# left pad of row r+1 (shared), so rows butt together seamlessly
PH = 58  # padded rows (0 and 57 are SAME-padding rows)
PPI = PH * PW  # 3306 padded pixels per image
GUARD_L = 1
GUARD_R = 5
PLANE = GUARD_L + PPI + GUARD_R  # 3312 bytes per partition per image
OUT0 = GUARD_L + PW + 1  # plane offset of first output pixel (row1,col1) = 59
NPX = 7 * 456  # 3192: px run covering out rows 1..56 (56 rows x 57)
BLKS = [456] * 7  # 7 blocks of exactly 8 padded rows each (456 = 8*57)
F32 = mybir.dt.float32
F16 = mybir.dt.float16
FP8 = mybir.dt.float8e4
U8 = mybir.dt.uint8

# tap (di,dj) reads plane offset p + PW*(di-1) + (dj-1) for output pixel p.
# DoubleRow pairs (first-tap shift, delta) + leftover single tap.
PAIRS = [  # ((di,dj) k=0, (di,dj) k=1)
    ((0, 0), (0, 1)),
    ((1, 0), (1, 1)),
    ((2, 0), (2, 1)),
    ((0, 2), (1, 2)),
]
SINGLE = (2, 2)


def _shift(t):
    return PW * (t[0] - 1) + (t[1] - 1)


def build_nc() -> bass.Bass:
    nc = bacc.Bacc()
    xp_t = nc.dram_tensor("xp", [IMGS, C, PLANE], U8, kind="ExternalInput")
    # [ci, 4 pairs * 2 * 256 + 256] = packed DR pairs + leftover tap
    w_t = nc.dram_tensor("w", [C, 9 * O], U8, kind="ExternalInput")
    sc_t = nc.dram_tensor("sc", [IMGS, 2, 128, NPX], F16, kind="ExternalOutput")

    with TileContext(nc) as tc:
        with (
            tc.tile_pool(name="const", bufs=1) as constp,
            tc.tile_pool(name="xin", bufs=4) as xinp,
            tc.tile_pool(name="stage", bufs=3) as stagep,
            tc.tile_pool(name="psum", bufs=7, space="PSUM") as psump,
            tc.tile_pool(name="warmps", bufs=1, space="PSUM") as warmpsp,
        ):
            wt = constp.tile([C, 9 * O], U8)
            nc.sync.dma_start(out=wt[:], in_=w_t[:])
            w8 = wt[:].bitcast(FP8)

            # PE warmup while input DMAs land: dummy matmuls on a zeroed
            # tile get HAM to K=8/8 before the real stream starts.
            warm = constp.tile([C, 512], U8)
            nc.vector.memset(warm[:], 0)
            wps = warmpsp.tile([128, 512], F32)
            for _ in range(10):
                nc.tensor.matmul(
                    wps[:, :], warm[:, :128].bitcast(FP8),
                    warm[:].bitcast(FP8), start=True, stop=True,
                )

            def w_pair(p, h):
                # lhsT [128, 2, 128]: planes at step O, co-half offset
                a = w8[:, p * 2 * O + h * 128 :]
                return AP(a.tensor, a.offset, [list(a.ap[0]), [O, 2], [1, 128]])

            def w_single(h):
                return w8[:, 8 * O + h * 128 : 8 * O + h * 128 + 128]

            HALF_P = PLANE // 2
            xtiles = {}
            for i in range(IMGS):
                xt = xinp.tile([C, PLANE], U8)
                # split each image across the two HWDGE queues
                nc.sync.dma_start(out=xt[:, :HALF_P], in_=xp_t[i][:, :HALF_P])
                nc.scalar.dma_start(out=xt[:, HALF_P:], in_=xp_t[i][:, HALF_P:])
                xtiles[i] = xt

            for i in range(IMGS):
                x8 = xtiles[i][:].bitcast(FP8)
                pstep = list(x8.ap[0])

                def rhs_pair(pair, base, n):
                    s0 = _shift(pair[0])
                    d = _shift(pair[1]) - s0
                    return AP(
                        x8.tensor, x8.offset + base + s0, [pstep, [d, 2], [1, n]]
                    )

                # output DMA chunks; finer-grained for the last image so the
                # SWDGE ring is nearly drained when the final block finishes
                if i == IMGS - 1:
                    CHUNKS = [(b, b * 456, (b + 1) * 456) for b in range(7)]
                else:
                    CHUNKS = [(3, 0, 4 * 456), (5, 4 * 456, 6 * 456), (6, 6 * 456, NPX)]
                for h in range(2):
                    stage = stagep.tile([128, NPX], F16)
                    off = 0
                    for bi, n in enumerate(BLKS):
                        base = OUT0 + off
                        ps = psump.tile([128, 512], F32)
                        for p in range(4):
                            nc.tensor.matmul(
                                ps[:, :n],
                                w_pair(p, h),
                                rhs_pair(PAIRS[p], base, n),
                                start=(p == 0),
                                stop=False,
                                perf_mode=mybir.MatmulPerfMode.DoubleRow,
                            )
                        nc.tensor.matmul(
                            ps[:, :n],
                            w_single(h),
                            AP(
                                x8.tensor,
                                x8.offset + base + _shift(SINGLE),
                                [pstep, [1, n]],
                            ),
                            start=False,
                            stop=True,
                        )
                        nc.vector.tensor_copy(stage[:, off : off + n], ps[:, :n])
                        off += n
                        # last images' outputs go via the (idle, HWDGE) sync
                        # ring so the SWDGE ring + its slow drain finish early
                        oq = nc.sync if i == IMGS - 1 else nc.gpsimd
                        for cb, clo, chi in CHUNKS:
                            if bi == cb:
                                oq.dma_start(
                                    out=sc_t[i, h][:, clo:chi],
                                    in_=stage[:, clo:chi],
                                )

    nc.finalize()
    return nc


_NC_CACHE = None


def _get_nc():
    global _NC_CACHE
    if _NC_CACHE is None:
        _NC_CACHE = build_nc()
    return _NC_CACHE


def prep_x(x: np.ndarray) -> np.ndarray:
    """(64,56,56,128) f32 -> (64,128,PLANE) u8 fp8e4 +-1, zero-padded."""
    xu = np.where(x >= 0, np.uint8(0x38), np.uint8(0xB8))
    planes = np.zeros((x.shape[0], C, PLANE), dtype=np.uint8)
    view = planes[:, :, GUARD_L : GUARD_L + PPI].reshape(x.shape[0], C, PH, PW)
    view[:, :, 1 : 1 + H, 1 : 1 + W] = xu.transpose(0, 3, 1, 2)
    return planes


def prep_w(w: np.ndarray) -> np.ndarray:
    """(3,3,128,256) f32 -> [128, 9*256] u8: 4 DR pairs [p,k,co] + single."""
    wu = np.where(w >= 0, np.uint8(0x38), np.uint8(0xB8))  # [3,3,ci,co]
    out = np.empty((C, 9 * O), dtype=np.uint8)
    for p, (t0, t1) in enumerate(PAIRS):
        out[:, p * 2 * O : p * 2 * O + O] = wu[t0[0], t0[1]]
        out[:, p * 2 * O + O : p * 2 * O + 2 * O] = wu[t1[0], t1[1]]
    out[:, 8 * O :] = wu[SINGLE[0], SINGLE[1]]
    return np.ascontiguousarray(out)


def _ntff_hook():
    sys.path.insert(0, "/root/.axon_site")
    from trn_agent_boot.trn_boot import _ntff_profile_via_ctypes

    return _ntff_profile_via_ctypes("/opt/axon/libaxon_pjrt.so")


def run(inputs: dict, profile_dir: str | None = None):
    """Run on all 8 NeuronCores. Returns (full_output, BassKernelResults)."""
    x = np.asarray(inputs["x"], dtype=np.float32)
    w = np.asarray(inputs["w"], dtype=np.float32)
    assert x.shape == (N_CORES * IMGS, H, W, C), x.shape
    assert w.shape == (3, 3, C, O), w.shape

    nc = _get_nc()
    planes = prep_x(x)
    wp = prep_w(w)
    in_maps = [
        {"xp": planes[i * IMGS : (i + 1) * IMGS], "w": wp} for i in range(N_CORES)
    ]
    if profile_dir is not None:
        hook = _ntff_hook()
        with hook(profile_dir, [0]):
            res = run_bass_kernel_spmd(nc, in_maps, list(range(N_CORES)))
    else:
        res = run_bass_kernel_spmd(nc, in_maps, list(range(N_CORES)))

    # [cores, img, half, co128, NPX] f16 -> (64,56,56,256) f32
    sc = np.stack([res.results[i]["sc"] for i in range(N_CORES)])
    v = sc.reshape(N_CORES, IMGS, 2, 128, H, PW)[..., :W]
    out = (
        v.transpose(0, 1, 4, 5, 2, 3)
        .astype(np.float32)
        .reshape(N_CORES * IMGS, H, W, O)
    )
    return out, res


def kernel(**inputs: np.ndarray) -> np.ndarray:
    out, _ = run(inputs)
    return out


# revision 3
# speedup vs baseline: 1.0060x; 1.0060x over previous
"""BinaryConv2D Trainium2 kernel — fp8 DoubleRow edition.

Reference computation:
    out = conv2d(sign(x), sign(w), SAME, stride 1)   # sign(v) = +1 if v>=0 else -1
    x: (64, 56, 56, 128) f32, w: (3, 3, 128, 256) f32 -> out (64, 56, 56, 256) f32

Strategy (data-parallel over batch, 8 images per NeuronCore):
  * Host binarizes x to fp8e4 +-1 bytes and lays it out channel-major into
    zero-padded 58x57 planes (57-wide rows share the left/right pad column;
    1B/5B guards), so the device does no input preprocessing at all: one
    straight DMA per image into SBUF (split across both HWDGE queues).
  * Weights are host-binarized to fp8 and packed as 4 DoubleRow tap-pairs
    [ci, pair, k, co] plus one leftover tap. A DoubleRow matmul contracts
    128 channels x 2 taps per pass: lhsT = w-pair [128, 2, 128co], rhs =
    two overlapping shifted windows of the padded plane [128, 2, Npx]
    (overlapping strided APs are legal on the moving operand), out PSUM
    [128co, Npx] f32. 9 taps = 4 DR + 1 normal matmul per 456-px block
    (7 blocks of 8 rows per image-half), DR first (normal->DR order in an
    accumulation group crashes the PE).
  * Output stays co-major: PSUM -> SBUF f16 via DVE (exact: integer sums
    <= 1152) -> HBM scratch [img, half, co128, px] on the gpsimd SWDGE
    ring (last image via the sync ring so the SWDGE drain overlaps the
    stream). The host strips row padding, transposes to NHWC and upcasts
    to f32 during unshard. 10 warmup matmuls on a zeroed tile cover the
    ~3.4us HAM cold-clock ramp while the first image's DMA lands.
  * Engine/queue budget is deliberately lopsided: heavy DMA stays on the
    single gpsimd ring and ScalarE stays near-idle — spreading output DMA
    over the sync/scalar HWDGE rings or running the PSUM evacuation on
    ScalarE reproducibly drops the whole chip from 2.4 to 2.0 GHz.
"""

import sys

if "/opt/trn_rl_repo" not in sys.path:
    sys.path.insert(0, "/opt/trn_rl_repo")

import numpy as np

import concourse.bacc as bacc
import concourse.bass as bass
import concourse.mybir as mybir
from concourse.ap import AP
from concourse.tile import TileContext
from concourse.bass_utils import run_bass_kernel_spmd

N_CORES = 8
IMGS = 8  # images per core
H = W = 56
C = 128  # input channels (contraction partitions)
O = 256  # output channels
PW = 57  # padded row width: [pad, 56 data] — the right pad of row r IS the
# left pad of row r+1 (shared), so rows butt together seamlessly
PH = 58  # padded rows (0 and 57 are SAME-padding rows)
PPI = PH * PW  # 3306 padded pixels per image
GUARD_L = 1
GUARD_R = 5
PLANE = GUARD_L + PPI + GUARD_R  # 3312 bytes per partition per image
OUT0 = GUARD_L + PW + 1  # plane offset of first output pixel (row1,col1) = 59
NPX = 7 * 456  # 3192: px run covering out rows 1..56 (56 rows x 57)
BLKS = [456] * 7  # 7 blocks of exactly 8 padded rows each (456 = 8*57)
F32 = mybir.dt.float32
F16 = mybir.dt.float16
FP8 = mybir.dt.float8e4
U8 = mybir.dt.uint8

# tap (di,dj) reads plane offset p + PW*(di-1) + (dj-1) for output pixel p.
# DoubleRow pairs (first-tap shift, delta) + leftover single tap.
PAIRS = [  # ((di,dj) k=0, (di,dj) k=1)
    ((0, 0), (0, 1)),
    ((1, 0), (1, 1)),
    ((2, 0), (2, 1)),
    ((0, 2), (1, 2)),
]
SINGLE = (2, 2)


def _shift(t):
    return PW * (t[0] - 1) + (t[1] - 1)


def build_nc() -> bass.Bass:
    nc = bacc.Bacc()
    xp_t = nc.dram_tensor("xp", [IMGS, C, PLANE], U8, kind="ExternalInput")
    # [ci, 4 pairs * 2 * 256 + 256] = packed DR pairs + leftover tap
    w_t = nc.dram_tensor("w", [C, 9 * O], U8, kind="ExternalInput")
    sc_t = nc.dram_tensor("sc", [IMGS, 2, 128, NPX], F16, kind="ExternalOutput")

    with TileContext(nc) as tc:
        with (
            tc.tile_pool(name="const", bufs=1) as constp,
            tc.tile_pool(name="xin", bufs=4) as xinp,
            tc.tile_pool(name="stage", bufs=3) as stagep,
            tc.tile_pool(name="psum", bufs=7, space="PSUM") as psump,
            tc.tile_pool(name="warmps", bufs=1, space="PSUM") as warmpsp,
        ):
            wt = constp.tile([C, 9 * O], U8)
            nc.sync.dma_start(out=wt[:], in_=w_t[:])
            w8 = wt[:].bitcast(FP8)

            # PE warmup while input DMAs land: dummy matmuls on a zeroed
            # tile get HAM to K=8/8 before the real stream starts.
            warm = constp.tile([C, 512], U8)
            nc.vector.memset(warm[:], 0)
            wps = warmpsp.tile([128, 512], F32)
            for _ in range(10):
                nc.tensor.matmul(
                    wps[:, :], warm[:, :128].bitcast(FP8),
                    warm[:].bitcast(FP8), start=True, stop=True,
                )

            def w_pair(p, h):
                # lhsT [128, 2, 128]: planes at step O, co-half offset
                a = w8[:, p * 2 * O + h * 128 :]
                return AP(a.tensor, a.offset, [list(a.ap[0]), [O, 2], [1, 128]])

            def w_single(h):
                return w8[:, 8 * O + h * 128 : 8 * O + h * 128 + 128]

            HALF_P = PLANE // 2
            xtiles = {}
            for i in range(IMGS):
                xt = xinp.tile([C, PLANE], U8)
                # split each image across the two HWDGE queues
                nc.sync.dma_start(out=xt[:, :HALF_P], in_=xp_t[i][:, :HALF_P])
                nc.scalar.dma_start(out=xt[:, HALF_P:], in_=xp_t[i][:, HALF_P:])
                xtiles[i] = xt

            for i in range(IMGS):
                x8 = xtiles[i][:].bitcast(FP8)
                pstep = list(x8.ap[0])

                def rhs_pair(pair, base, n):
                    s0 = _shift(pair[0])
                    d = _shift(pair[1]) - s0
                    return AP(
                        x8.tensor, x8.offset + base + s0, [pstep, [d, 2], [1, n]]
                    )

                # output DMA chunks; finer-grained for the last image so the
                # SWDGE ring is nearly drained when the final block finishes
                if i == IMGS - 1:
                    CHUNKS = [(b, b * 456, (b + 1) * 456) for b in range(7)]
                else:
                    CHUNKS = [(3, 0, 4 * 456), (5, 4 * 456, 6 * 456), (6, 6 * 456, NPX)]
                for h in range(2):
                    stage = stagep.tile([128, NPX], F16)
                    off = 0
                    for bi, n in enumerate(BLKS):
                        base = OUT0 + off
                        ps = psump.tile([128, 512], F32)
                        for p in range(4):
                            nc.tensor.matmul(
                                ps[:, :n],
                                w_pair(p, h),
                                rhs_pair(PAIRS[p], base, n),
                                start=(p == 0),
                                stop=False,
                                perf_mode=mybir.MatmulPerfMode.DoubleRow,
                            )
                        nc.tensor.matmul(
                            ps[:, :n],
                            w_single(h),
                            AP(
                                x8.tensor,
                                x8.offset + base + _shift(SINGLE),
                                [pstep, [1, n]],
                            ),
                            start=False,
                            stop=True,
                        )
                        nc.vector.tensor_copy(stage[:, off : off + n], ps[:, :n])
                        off += n
                        # last images' outputs go via the (idle, HWDGE) sync
                        # ring so the SWDGE ring + its slow drain finish early
                        oq = nc.sync if i == IMGS - 1 else nc.gpsimd
                        for cb, clo, chi in CHUNKS:
                            if bi == cb:
                                oq.dma_start(
                                    out=sc_t[i, h][:, clo:chi],
                                    in_=stage[:, clo:chi],
                                )

    nc.finalize()
    return nc


_NC_CACHE = None


def _get_nc():
    global _NC_CACHE
    if _NC_CACHE is None:
        _NC_CACHE = build_nc()
    return _NC_CACHE


def prep_x(x: np.ndarray) -> np.ndarray:
    """(64,56,56,128) f32 -> (64,128,PLANE) u8 fp8e4 +-1, zero-padded."""
    xu = np.where(x >= 0, np.uint8(0x38), np.uint8(0xB8))
    planes = np.zeros((x.shape[0], C, PLANE), dtype=np.uint8)
    view = planes[:, :, GUARD_L : GUARD_L + PPI].reshape(x.shape[0], C, PH, PW)
    view[:, :, 1 : 1 + H, 1 : 1 + W] = xu.transpose(0, 3, 1, 2)
    return planes


def prep_w(w: np.ndarray) -> np.ndarray:
    """(3,3,128,256) f32 -> [128, 9*256] u8: 4 DR pairs [p,k,co] + single."""
    wu = np.where(w >= 0, np.uint8(0x38), np.uint8(0xB8))  # [3,3,ci,co]
    out = np.empty((C, 9 * O), dtype=np.uint8)
    for p, (t0, t1) in enumerate(PAIRS):
        out[:, p * 2 * O : p * 2 * O + O] = wu[t0[0], t0[1]]
        out[:, p * 2 * O + O : p * 2 * O + 2 * O] = wu[t1[0], t1[1]]
    out[:, 8 * O :] = wu[SINGLE[0], SINGLE[1]]
    return np.ascontiguousarray(out)


def _ntff_hook():
    sys.path.insert(0, "/root/.axon_site")
    from trn_agent_boot.trn_boot import _ntff_profile_via_ctypes

    return _ntff_profile_via_ctypes("/opt/axon/libaxon_pjrt.so")


def run(inputs: dict, profile_dir: str | None = None):
    """Run on all 8 NeuronCores. Returns (full_output, BassKernelResults)."""
    x = np.asarray(inputs["x"], dtype=np.float32)
    w = np.asarray(inputs["w"], dtype=np.float32)
    assert x.shape == (N_CORES * IMGS, H, W, C), x.shape
    assert w.shape == (3, 3, C, O), w.shape

    nc = _get_nc()
    planes = prep_x(x)
    wp = prep_w(w)
    in_maps = [
        {"xp": planes[i * IMGS : (i + 1) * IMGS], "w": wp} for i in range(N_CORES)
    ]
    if profile_dir is not None:
        hook = _ntff_hook()
        with hook(profile_dir, [0]):
            res = run_bass_kernel_spmd(nc, in_maps, list(range(N_CORES)))
    else:
        res = run_bass_kernel_spmd(nc, in_maps, list(range(N_CORES)))

    # [cores, img, half, co128, NPX] f16 -> (64,56,56,256) f32
    sc = np.stack([res.results[i]["sc"] for i in range(N_CORES)])
    v = sc.reshape(N_CORES, IMGS, 2, 128, H, PW)[..., :W]
    out = (
        v.transpose(0, 1, 4, 5, 2, 3)
        .astype(np.float32)
        .reshape(N_CORES * IMGS, H, W, O)
    )
    return out, res


def kernel(**inputs: np.ndarray) -> np.ndarray:
    out, _ = run(inputs)
    return out
